# revision 8
# baseline (speedup 1.0000x reference)
"""Trainium2 Bass kernel for LISTA (nn_LISTA_37976100831401).

Data-parallel sharding: batch 16384 -> 8 NeuronCores x 2048 rows.
We / S / theta are replicated on every core; there is no cross-device
communication in the forward pass.

Per-core algorithm:
  B  = X @ We.T                 (2048, 1024)
  Z0 = soft(B);  Z_{t+1} = soft(B + Z_t @ S.T), t = 0..15
  soft(x) = x - clip(x, -theta, +theta)

Layout: keep the feature dim m=1024 on SBUF partitions (8 tiles of 128) and
batch on the free dim.  Then C.T = S @ Z.T + B.T accumulates in PSUM via
  psum[j,b] += ST[k][:, j128].T @ ZT[k][:, b]        (lhsT = S.T tile)
and the matmul OUTPUT layout [j, b] equals the INPUT layout [k, b] the next
step needs -- no per-step transposes.  All matmuls run in fp32r (1 PE
cycle/row for moving dim >= 256, vs 4 cycles/row for plain fp32).

The final step is computed in the flipped orientation [b, j] (stationary =
Z.T columns, moving = S.T) with the X@We.T term accumulated in the same PSUM
group, so the result lands batch-major and DMAs out contiguously.
"""

from contextlib import ExitStack

import numpy as np

import concourse.bacc as bacc
import concourse.mybir as mybir
import concourse.tile as tile
from concourse import bass_utils
from concourse.masks import make_identity

FP32 = mybir.dt.float32
FP32R = mybir.dt.float32r
AL = mybir.AluOpType
AF = mybir.ActivationFunctionType

N_CORES = 8
B_TOTAL, N_IN, M = 16384, 256, 1024
B_CORE = B_TOTAL // N_CORES  # 2048
T_STEPS = 16                 # scan length in the reference
CHUNK = 512                  # batch columns per j-sweep (= PSUM bank / max N)
N_CHUNKS = B_CORE // CHUNK   # 4
KT = M // 128                # 8 feature tiles of 128
NT = N_IN // 128             # 2 input-feature tiles
JHN = M // CHUNK             # 2 output-column halves in the final step


def _emit(ctx: ExitStack, tc: tile.TileContext, X, We, S, theta, Z):
    nc = tc.nc

    const_pool = ctx.enter_context(tc.tile_pool(name="const", bufs=1))
    psum_pool = ctx.enter_context(tc.tile_pool(name="psum", bufs=1, space="PSUM"))
    load_pool = ctx.enter_context(tc.tile_pool(name="load", bufs=1))
    xt_pool = ctx.enter_context(tc.tile_pool(name="xt", bufs=1))
    bt_pool = ctx.enter_context(tc.tile_pool(name="bt", bufs=1))
    zt_pool = ctx.enter_context(tc.tile_pool(name="zt", bufs=1))
    tmp_pool = ctx.enter_context(tc.tile_pool(name="tmp", bufs=1))
    out_pool = ctx.enter_context(tc.tile_pool(name="zout", bufs=1))

    # ---- constants -------------------------------------------------------
    ident = const_pool.tile([128, 128], FP32, name="ident")
    make_identity(nc, ident[:])

    th = const_pool.tile([128, KT], FP32, name="th")
    nth = const_pool.tile([128, KT], FP32, name="nth")
    wet = [const_pool.tile([128, M], FP32R, name=f"wet{nt}") for nt in range(NT)]
    st = [const_pool.tile([128, M], FP32R, name=f"st{kt}") for kt in range(KT)]
    thbc = const_pool.tile([128, M], FP32, name="thbc")
    nthbc = const_pool.tile([128, M], FP32, name="nthbc")

    def emit_th():
        # theta as per-partition columns: th[p, jt] = theta[jt*128 + p]
        for jt in range(KT):
            nc.sync.dma_start(
                th[:, jt : jt + 1],
                theta[jt * 128 : (jt + 1) * 128].rearrange("(p o) -> p o", o=1),
            )
        nc.vector.tensor_scalar_mul(nth[:], th[:], -1.0)

    def emit_we():
        # We.T tiles: wet[nt][n_p, j] = We[j, nt*128 + n_p]
        for jt in range(KT):
            we_nat = load_pool.tile(
                [128, N_IN], FP32, name="we_nat", tag="we_nat", bufs=2
            )
            nc.sync.dma_start(we_nat[:], We[jt * 128 : (jt + 1) * 128, :])
            for nt in range(NT):
                pt = psum_pool.tile([128, 128], FP32, name="ptw", tag="tp", bufs=2)
                nc.tensor.transpose(
                    pt[:], we_nat[:, nt * 128 : (nt + 1) * 128], ident[:]
                )
                nc.vector.tensor_copy(wet[nt][:, jt * 128 : (jt + 1) * 128], pt[:])

    def emit_st():
        # S.T tiles: st[kt][k_p, j] = S[j, kt*128 + k_p]
        for jt in range(KT):
            s_nat = load_pool.tile([128, M], FP32, name="s_nat", tag="s_nat", bufs=2)
            nc.sync.dma_start(s_nat[:], S[jt * 128 : (jt + 1) * 128, :])
            for kt in range(KT):
                pts = psum_pool.tile([128, 128], FP32, name="pts", tag="tp", bufs=2)
                nc.tensor.transpose(
                    pts[:], s_nat[:, kt * 128 : (kt + 1) * 128], ident[:]
                )
                nc.vector.tensor_copy(st[kt][:, jt * 128 : (jt + 1) * 128], pts[:])

    def emit_thbc():
        # theta broadcast across partitions (for the flipped final step):
        # thbc[p, j] = theta[j], built with a K=1 ones-matmul.
        th_row = const_pool.tile([1, M], FP32, name="th_row")
        nc.sync.dma_start(th_row[:], theta.rearrange("(o m) -> o m", o=1))
        ones_col = const_pool.tile([1, 128], FP32, name="ones_col")
        nc.gpsimd.memset(ones_col[:], 1.0)
        for jh in range(JHN):
            pbc = psum_pool.tile([128, CHUNK], FP32, name="pbc", tag="tp", bufs=2)
            nc.tensor.matmul(
                pbc[:], ones_col[:], th_row[:, jh * CHUNK : (jh + 1) * CHUNK],
                start=True, stop=True,
            )
            nc.vector.tensor_copy(thbc[:, jh * CHUNK : (jh + 1) * CHUNK], pbc[:])
        nc.vector.tensor_scalar_mul(nthbc[:], thbc[:], -1.0)

    # ---- per-chunk state -------------------------------------------------
    xts = {}  # chunk -> [NT] tiles [128, CHUNK]   (X.T slab)
    bts = {}  # chunk -> [KT] tiles [128, CHUNK]   (B.T slab)
    zts = {}  # chunk -> [KT] tiles [128, CHUNK]   (current Z.T)

    def x_phase(c):
        row0 = c * CHUNK
        xts[c] = [
            xt_pool.tile([128, CHUNK], FP32R, name=f"xt{nt}", tag=f"xt{nt}", bufs=4)
            for nt in range(NT)
        ]
        for bt in range(CHUNK // 128):
            xn = load_pool.tile([128, N_IN], FP32, name="xn", tag="xn", bufs=3)
            nc.sync.dma_start(xn[:], X[row0 + bt * 128 : row0 + (bt + 1) * 128, :])
            for nt in range(NT):
                ptx = psum_pool.tile([128, 128], FP32, name="ptx", tag="tp", bufs=2)
                nc.tensor.transpose(ptx[:], xn[:, nt * 128 : (nt + 1) * 128], ident[:])
                nc.vector.tensor_copy(
                    xts[c][nt][:, bt * 128 : (bt + 1) * 128], ptx[:]
                )

    def b_phase(c):
        # B.T = We @ X.T ; Z0 = soft(B)
        bts[c] = []
        zts[c] = []
        for jt in range(KT):
            ps = psum_pool.tile([128, CHUNK], FP32, name="psb", tag="mm", bufs=6)
            for nt in range(NT):
                nc.tensor.matmul(
                    ps[:],
                    wet[nt][:, jt * 128 : (jt + 1) * 128],
                    xts[c][nt][:],
                    start=(nt == 0),
                    stop=(nt == NT - 1),
                )
            btile = bt_pool.tile(
                [128, CHUNK], FP32, name="btile", tag=f"bt{jt}", bufs=2
            )
            nc.vector.tensor_copy(btile[:], ps[:])
            af = tmp_pool.tile([128, CHUNK], FP32, name="afb", tag="af", bufs=3)
            nc.scalar.activation(
                af[:], ps[:], AF.Relu, bias=nth[:, jt : jt + 1], scale=1.0
            )
            df = tmp_pool.tile([128, CHUNK], FP32, name="dfb", tag="df", bufs=3)
            nc.scalar.activation(
                df[:], ps[:], AF.Relu, bias=nth[:, jt : jt + 1], scale=-1.0
            )
            z0 = zt_pool.tile([128, CHUNK], FP32R, name="z0", tag=f"zt{jt}", bufs=3)
            nc.vector.tensor_sub(z0[:], af[:], df[:])
            bts[c].append(btile)
            zts[c].append(z0)

    def step(c):
        # Z <- soft(B + Z @ S.T), in the [j, b] orientation.
        zcur = zts[c]
        znew = []
        for jt in range(KT):
            ps = psum_pool.tile([128, CHUNK], FP32, name="pss", tag="mm", bufs=6)
            for kt in range(KT):
                nc.tensor.matmul(
                    ps[:],
                    st[kt][:, jt * 128 : (jt + 1) * 128],
                    zcur[kt][:],
                    start=(kt == 0),
                    stop=(kt == KT - 1),
                )
            ct = tmp_pool.tile([128, CHUNK], FP32, name="ct", tag="ct", bufs=3)
            nc.vector.tensor_add(ct[:], ps[:], bts[c][jt][:])
            af = tmp_pool.tile([128, CHUNK], FP32, name="afs", tag="af", bufs=3)
            nc.scalar.activation(
                af[:], ct[:], AF.Relu, bias=nth[:, jt : jt + 1], scale=1.0
            )
            df = tmp_pool.tile([128, CHUNK], FP32, name="dfs", tag="df", bufs=3)
            nc.scalar.activation(
                df[:], ct[:], AF.Relu, bias=nth[:, jt : jt + 1], scale=-1.0
            )
            zn = zt_pool.tile([128, CHUNK], FP32R, name="zn", tag=f"zt{jt}", bufs=3)
            nc.vector.tensor_sub(zn[:], af[:], df[:])
            znew.append(zn)
        zts[c] = znew

    def final_tile(c, idx):
        # Last step in flipped orientation: out[b, j], so the store DMA is
        # contiguous along DRAM rows.  C = Z@S.T + X@We.T accumulated in PSUM.
        row0 = c * CHUNK
        bt, jh = divmod(idx, JHN)
        zcur = zts[c]
        jsl = slice(jh * CHUNK, (jh + 1) * CHUNK)
        ps = psum_pool.tile([128, CHUNK], FP32, name="psf", tag="mm", bufs=6)
        for kt in range(KT):
            nc.tensor.matmul(
                ps[:],
                zcur[kt][:, bt * 128 : (bt + 1) * 128],
                st[kt][:, jsl],
                start=(kt == 0),
                stop=False,
            )
        for nt in range(NT):
            nc.tensor.matmul(
                ps[:],
                xts[c][nt][:, bt * 128 : (bt + 1) * 128],
                wet[nt][:, jsl],
                start=False,
                stop=(nt == NT - 1),
            )
        t1 = tmp_pool.tile([128, CHUNK], FP32, name="t1f", tag="ct", bufs=3)
        nc.vector.tensor_max(t1[:], ps[:], nthbc[:, jsl])
        t2 = tmp_pool.tile([128, CHUNK], FP32, name="t2f", tag="tcl", bufs=2)
        nc.vector.tensor_tensor(t2[:], t1[:], thbc[:, jsl], op=AL.min)
        zo = out_pool.tile([128, CHUNK], FP32, name="zo", tag="zo", bufs=3)
        nc.vector.tensor_sub(zo[:], ps[:], t2[:])
        nc.sync.dma_start(Z[row0 + bt * 128 : row0 + (bt + 1) * 128, jsl], zo[:])

    # ---- emission schedule: PE starts on X transposes almost immediately;
    # chunk pairs interleave at step granularity so the PE never stalls on a
    # step's soft-threshold tail; pair 1's X loads prefetch before pair 0's
    # final steps.
    x_phase(0)
    x_phase(1)
    emit_we()
    emit_th()
    b_phase(0)
    b_phase(1)
    emit_st()
    emit_thbc()
    for _ in range(T_STEPS - 1):
        step(0)
        step(1)
    x_phase(2)
    x_phase(3)
    b_phase(2)
    b_phase(3)
    for idx in range(CHUNK // 128 * JHN):
        final_tile(0, idx)
        final_tile(1, idx)
    for _ in range(T_STEPS - 1):
        step(2)
        step(3)
    for idx in range(CHUNK // 128 * JHN):
        final_tile(2, idx)
        final_tile(3, idx)


def build_nc():
    nc = bacc.Bacc("TRN2", target_bir_lowering=False, debug=False)
    X = nc.dram_tensor("X", [B_CORE, N_IN], FP32, kind="ExternalInput")
    We = nc.dram_tensor("We", [M, N_IN], FP32, kind="ExternalInput")
    S = nc.dram_tensor("S", [M, M], FP32, kind="ExternalInput")
    theta = nc.dram_tensor("theta", [M], FP32, kind="ExternalInput")
    Z = nc.dram_tensor("Z", [B_CORE, M], FP32, kind="ExternalOutput")
    with tile.TileContext(nc) as tc:
        with ExitStack() as ctx:
            _emit(ctx, tc, X.ap(), We.ap(), S.ap(), theta.ap(), Z.ap())
    nc.compile()
    return nc


_NC_CACHE = None


def _get_nc():
    global _NC_CACHE
    if _NC_CACHE is None:
        _NC_CACHE = build_nc()
    return _NC_CACHE


def run(X, We, S, theta, trace=False, **trace_kwargs):
    nc = _get_nc()
    X = np.ascontiguousarray(np.asarray(X, dtype=np.float32))
    We = np.ascontiguousarray(np.asarray(We, dtype=np.float32))
    S = np.ascontiguousarray(np.asarray(S, dtype=np.float32))
    theta = np.ascontiguousarray(np.asarray(theta, dtype=np.float32))
    in_maps = [
        {
            "X": np.ascontiguousarray(X[i * B_CORE : (i + 1) * B_CORE]),
            "We": We,
            "S": S,
            "theta": theta,
        }
        for i in range(N_CORES)
    ]
    res = bass_utils.run_bass_kernel_spmd(
        nc, in_maps, list(range(N_CORES)), trace=trace, **trace_kwargs
    )
    Z = np.concatenate([res.results[i]["Z"] for i in range(N_CORES)], axis=0)
    return Z.astype(np.float32, copy=False), res


def kernel(X, We, S, theta):
    Z, _ = run(X, We, S, theta, trace=False)
    return Z
